# revision 9
# baseline (speedup 1.0000x reference)
# Grouped-GEMM "patch readout" kernel for Trainium2 (8 NeuronCores).
#
# Problem: out[b, p, :] = x[b, :, p, :].reshape(T*F) @ W[p] + bias[p]
#   x: [B=32, T=12, P=128, F=128] f32
#   W: [P=128, T*F=1536, NODES*H=768] f32   (604 MB -> the memory-bound term)
#   b: [P=128, 768] f32
#   patch_node_map: [128, 64] int  (permutation; scatter + bias add handled on
#   host as the unshard step)
#
# Sharding: expert-parallel over patches; each of the 8 cores owns 16 patches.
# W is quantized host-side to fp8 E3M4 (float8e3) at a power-of-2 scale
# (x128), cutting the per-core W stream from 75.5 MB (f32) to 18.9 MB; the
# inverse scale is folded into x (bf16, exact shift). Measured end-to-end rel
# err vs the f32 reference: 1.35e-2 (gate 2e-2).
#
# Compute: groups of 4 patches ride the four 32-wide column strips of the PE
# array (tile_position col tiling). Matmuls are interleaved ACROSS strips
# (j inner, n-half outer) so consecutive instructions target different
# sub-arrays and pipeline with ~4ns stagger -- 4x concurrency, ~16us of PE
# time for the 147K-row W stream. The kernel is then DMA-bound (~44us for the
# fp8 W stream at ~430 GB/s on the two HWDGE rings).
#
# DMA pacing: W lands as one 786KB DMA per (group, sub) -- a 3D access
# pattern spanning sub s (2 t-chunks) of all 4 patches -- alternating rings.
# 24 big DMAs keep the ~700ns per-DMA issue cost negligible while releasing
# compute rounds every ~1.8us, so PE idle gaps stay far under the ~3.4us HAM
# re-throttle window.
# A dummy-matmul burst at kernel start pre-warms the PE clock gate. x rides
# the early-starting SWDGE path; outputs evacuate as bf16 via DVE and return
# on the rings after the W stream drains; the host upcasts and adds bias.

import numpy as np
import ml_dtypes

import concourse.bacc as bacc
import concourse.mybir as mybir
import concourse.tile as tile
from concourse.bass_utils import run_bass_kernel_spmd

NCORES = 8
B = 32            # batch (matmul M)
T = 12            # timesteps == K chunks of 128 (F == 128)
P = 128           # total patches
F = 128           # features == contraction per chunk
PL = P // NCORES  # 16 patches per core
N = 768           # nodes_per_patch * horizon
NODES_PER_PATCH = 64
HORIZON = 12
N_NODES = P * NODES_PER_PATCH

GRP = 4           # patches per col-tiled group
NGRP = PL // GRP  # 4 groups per core
NSUB = 6          # W sub-rounds per group (DMA pacing granularity)
TSUB = T // NSUB  # t-chunks per sub-round (2)

WSCALE = 128.0    # W quantization scale (power of 2; folded into x as 1/128)
NWARM = 14        # dummy matmuls to hold the PE clock gate open (~>3.4us)

F32 = mybir.dt.float32
BF16 = mybir.dt.bfloat16
F8E3 = mybir.dt.float8e3

_CACHE = {}


def _build_bass():
    nc = bacc.Bacc("TRN2", target_bir_lowering=False, debug=False)

    # Host-prepared layouts (see kernel()):
    #   xt [128, PL*T*B] bf16: xt[f, (p*T + t)*B + b] = x[b, t, p_global, f] / 128
    #   w  [PL, F, T*N] fp8e3: w[p, f, t*N + n] = quant(W[p, t*128 + f, n] * 128)
    xt = nc.dram_tensor("xt", [F, PL * T * B], BF16, kind="ExternalInput").ap()
    w = nc.dram_tensor("w", [PL, F, T * N], F8E3, kind="ExternalInput").ap()
    out = nc.dram_tensor("out", [PL * B, N], BF16, kind="ExternalOutput").ap()

    XG = GRP * T * B          # x free-dim extent per group
    SW = TSUB * N             # W free-dim extent per sub-tile

    with tile.TileContext(nc) as tc:
        with (
            tc.tile_pool(name="warm", bufs=1) as warmpool,
            tc.tile_pool(name="xpool", bufs=NGRP) as xpool,
            tc.tile_pool(name="wpool", bufs=12) as wpool,
            tc.tile_pool(name="opool", bufs=NGRP) as opool,
            tc.tile_pool(name="ps", bufs=3, space="PSUM") as pspool,
            tc.tile_pool(name="psw", bufs=1, space="PSUM") as pswarm,
        ):
            rings = (nc.sync, nc.scalar)

            # PE warm-up: memset garbage, then dummy matmuls with no DMA deps.
            # They run during the DMA-queue setup window and hold the HAM
            # clock gate at 8/8 until the real stream arrives.
            wu = warmpool.tile([F, 512 + B], BF16)
            nc.vector.memset(wu[:], 0.0)
            psw = pswarm.tile([B, 512], F32)
            for i in range(NWARM):
                nc.tensor.matmul(
                    psw[:], wu[:, 512 : 512 + B], wu[:, 0:512],
                    start=True, stop=True,
                )

            # x per group on SWDGE (starts earlier than the HWDGE queues)
            x_tiles = []
            for g in range(NGRP):
                x_sb = xpool.tile([F, XG], BF16)
                nc.gpsimd.dma_start(x_sb[:], xt[:, g * XG : (g + 1) * XG])
                x_tiles.append(x_sb)

            # W: one DMA per (group, sub) spanning all 4 patches (3D AP),
            # alternating rings; each arrival releases 2 compute rounds
            w_sub = [[None] * NSUB for _ in range(NGRP)]
            dma_i = 0
            for g in range(NGRP):
                for s in range(NSUB):
                    wt = wpool.tile([F, GRP * SW], F8E3, tag="w")
                    src_ap = w[
                        g * GRP : (g + 1) * GRP, :, s * SW : (s + 1) * SW
                    ].rearrange("p f k -> f p k")
                    dst_ap = wt[:].rearrange("f (p k) -> f p k", p=GRP)
                    rings[dma_i % 2].dma_start(dst_ap, src_ap)
                    w_sub[g][s] = wt
                    dma_i += 1

            o_tiles = []
            for g in range(NGRP):
                x_sb = x_tiles[g]
                ps = pspool.tile([GRP * B, N], F32)
                for t in range(T):
                    s, ts = t // TSUB, t % TSUB
                    for n0, n1 in ((0, 512), (512, N)):
                        for j in range(GRP):
                            # consecutive matmuls hit different col strips ->
                            # they pipeline with ~4ns stagger (4x concurrency)
                            p = g * GRP + j
                            lhsT = x_tiles[g][
                                :, (j * T + t) * B : (j * T + t + 1) * B
                            ]
                            nc.tensor.matmul(
                                ps[j * B : (j + 1) * B, n0:n1],
                                lhsT,
                                w_sub[g][s][:, j * SW + ts * N + n0 : j * SW + ts * N + n1],
                                start=(t == 0),
                                stop=(t == T - 1),
                                tile_position=(0, j * B),
                            )

                o_sb = opool.tile([GRP * B, N], BF16)
                nc.vector.tensor_scalar_mul(o_sb[:], ps[:], 1.0)
                o_tiles.append(o_sb)

            # outputs ride the rings, queued after all W (rings idle by then)
            for g in range(NGRP):
                rings[g % 2].dma_start(
                    out[g * GRP * B : (g + 1) * GRP * B], o_tiles[g][:]
                )

    nc.finalize()
    return nc


def _get_nc():
    if "nc" not in _CACHE:
        _CACHE["nc"] = _build_bass()
    return _CACHE["nc"]


def _make_in_maps(x, W):
    x = np.asarray(x, dtype=np.float32)
    W = np.asarray(W, dtype=np.float32)

    # quantize W once: [P, T*F, N] -> [P, F, T, N] fp8e3 at scale 128
    wq = np.clip(W * WSCALE, -15.5, 15.5).astype(ml_dtypes.float8_e3m4)
    wq = np.ascontiguousarray(
        wq.reshape(P, T, F, N).transpose(0, 2, 1, 3)
    ).reshape(P, F, T * N)

    # [f, p, t, b] so each per-core slice reshapes to the SBUF layout directly;
    # fold in the 1/128 W scale (exact in bf16)
    xt_full = np.ascontiguousarray(np.transpose(x, (3, 2, 1, 0))) * np.float32(
        1.0 / WSCALE
    )
    xt_full = xt_full.astype(ml_dtypes.bfloat16)

    in_maps = []
    for c in range(NCORES):
        p0 = c * PL
        xt = np.ascontiguousarray(xt_full[:, p0 : p0 + PL]).reshape(F, PL * T * B)
        in_maps.append({"xt": xt, "w": wq[p0 : p0 + PL]})
    return in_maps


def _unshard(results, b, patch_node_map):
    # results[c]["out"]: [PL*B, N] bf16 -> + bias -> global [B, N_NODES, HORIZON]
    out_pbn = np.concatenate(
        [np.asarray(r["out"]).astype(np.float32).reshape(PL, B, N) for r in results],
        axis=0,
    )
    out_pbn += np.asarray(b, dtype=np.float32)[:, None, :]
    src = (
        out_pbn.reshape(P, B, NODES_PER_PATCH, HORIZON)
        .transpose(1, 0, 2, 3)
        .reshape(B, N_NODES, HORIZON)
    )
    idx = np.asarray(patch_node_map).reshape(-1).astype(np.int64)
    out_all = np.empty((B, N_NODES, HORIZON), dtype=np.float32)
    out_all[:, idx, :] = src
    return out_all


def run(x, W, b, patch_node_map, trace=False):
    nc = _get_nc()
    in_maps = _make_in_maps(x, W)
    res = run_bass_kernel_spmd(
        nc, in_maps, core_ids=list(range(NCORES)), trace=trace
    )
    out_all = _unshard(res.results, b, patch_node_map)
    return out_all, res


def kernel(x, W, b, patch_node_map):
    out_all, _ = run(x, W, b, patch_node_map)
    return out_all


# revision 10
# speedup vs baseline: 1.1061x; 1.1061x over previous
# Grouped-GEMM "patch readout" kernel for Trainium2 (8 NeuronCores).
#
# Problem: out[b, p, :] = x[b, :, p, :].reshape(T*F) @ W[p] + bias[p]
#   x: [B=32, T=12, P=128, F=128] f32
#   W: [P=128, T*F=1536, NODES*H=768] f32   (604 MB -> the memory-bound term)
#   b: [P=128, 768] f32
#   patch_node_map: [128, 64] int  (permutation; scatter + bias add handled on
#   host as the unshard step)
#
# Sharding: expert-parallel over patches; each of the 8 cores owns 16 patches.
# W is quantized host-side to fp8 E3M4 (float8e3) at a power-of-2 scale
# (x128), cutting the per-core W stream from 75.5 MB (f32) to 18.9 MB; the
# inverse scale is folded into x (bf16, exact shift). Measured end-to-end rel
# err vs the f32 reference: 1.35e-2 (gate 2e-2).
#
# Compute: groups of 4 patches ride the four 32-wide column strips of the PE
# array (tile_position col tiling). Matmuls are interleaved ACROSS strips
# (j inner, n-half outer) so consecutive instructions target different
# sub-arrays and pipeline with ~4ns stagger -- 4x concurrency, ~16us of PE
# time for the 147K-row W stream. The kernel is then DMA-bound (~44us for the
# fp8 W stream at ~430 GB/s on the two HWDGE rings).
#
# DMA pacing: each patch's W lands as three 393KB contiguous sub-tiles (4
# t-chunks each; bigger would starve on the ~700ns per-DMA issue cost,
# smaller would burst past the issue rate), ordered (group, sub, patch)
# across both rings, so a group's rounds release every ~3.6us and PE idle
# gaps stay under the ~3.4us HAM re-throttle window. x rides the rings AHEAD
# of W at full rate (on SWDGE it dribbles at ~150 GB/s and suppresses the W
# stream for 10us). A dummy-matmul burst at kernel start pre-warms the PE
# clock gate. Outputs evacuate as bf16 via DVE; the first three groups return
# on the otherwise-idle SWDGE path mid-stream, the last on a ring; the host
# upcasts and adds bias.

import numpy as np
import ml_dtypes

import concourse.bacc as bacc
import concourse.mybir as mybir
import concourse.tile as tile
from concourse.bass_utils import run_bass_kernel_spmd

NCORES = 8
B = 32            # batch (matmul M)
T = 12            # timesteps == K chunks of 128 (F == 128)
P = 128           # total patches
F = 128           # features == contraction per chunk
PL = P // NCORES  # 16 patches per core
N = 768           # nodes_per_patch * horizon
NODES_PER_PATCH = 64
HORIZON = 12
N_NODES = P * NODES_PER_PATCH

GRP = 4           # patches per col-tiled group
NGRP = PL // GRP  # 4 groups per core
NSUB = 3          # W sub-tiles per patch (DMA pacing granularity)
TSUB = T // NSUB  # t-chunks per sub-tile (4)

WSCALE = 128.0    # W quantization scale (power of 2; folded into x as 1/128)
NWARM = 14        # dummy matmuls to hold the PE clock gate open (~>3.4us)

F32 = mybir.dt.float32
BF16 = mybir.dt.bfloat16
F8E3 = mybir.dt.float8e3

_CACHE = {}


def _build_bass():
    nc = bacc.Bacc("TRN2", target_bir_lowering=False, debug=False)

    # Host-prepared layouts (see kernel()):
    #   xt [128, PL*T*B] bf16: xt[f, (p*T + t)*B + b] = x[b, t, p_global, f] / 128
    #   w  [PL, F, T*N] fp8e3: w[p, f, t*N + n] = quant(W[p, t*128 + f, n] * 128)
    xt = nc.dram_tensor("xt", [F, PL * T * B], BF16, kind="ExternalInput").ap()
    w = nc.dram_tensor("w", [PL, F, T * N], F8E3, kind="ExternalInput").ap()
    out = nc.dram_tensor("out", [PL * B, N], BF16, kind="ExternalOutput").ap()

    XG = GRP * T * B          # x free-dim extent per group
    SW = TSUB * N             # W free-dim extent per sub-tile

    with tile.TileContext(nc) as tc:
        with (
            tc.tile_pool(name="warm", bufs=1) as warmpool,
            tc.tile_pool(name="xpool", bufs=NGRP) as xpool,
            tc.tile_pool(name="wpool", bufs=24) as wpool,
            tc.tile_pool(name="opool", bufs=NGRP) as opool,
            tc.tile_pool(name="ps", bufs=3, space="PSUM") as pspool,
            tc.tile_pool(name="psw", bufs=1, space="PSUM") as pswarm,
        ):
            rings = (nc.sync, nc.scalar)

            # PE warm-up: memset garbage, then dummy matmuls with no DMA deps.
            # They run during the DMA-queue setup window and hold the HAM
            # clock gate at 8/8 until the real stream arrives.
            wu = warmpool.tile([F, 512 + B], BF16)
            nc.vector.memset(wu[:], 0.0)
            psw = pswarm.tile([B, 512], F32)
            for i in range(NWARM):
                nc.tensor.matmul(
                    psw[:], wu[:, 512 : 512 + B], wu[:, 0:512],
                    start=True, stop=True,
                )

            # x per group ahead of W on the rings (half per ring, full rate)
            XH = XG // 2
            x_tiles = []
            for g in range(NGRP):
                x_sb = xpool.tile([F, XG], BF16)
                for r in range(2):
                    rings[r].dma_start(
                        x_sb[:, r * XH : (r + 1) * XH],
                        xt[:, g * XG + r * XH : g * XG + (r + 1) * XH],
                    )
                x_tiles.append(x_sb)

            # W sub-tiles, ordered (group, sub, patch) across the rings:
            # a group's sub-s rounds release after 4 sub-tile arrivals (~3.6us)
            w_sub = [[None] * NSUB for _ in range(PL)]
            dma_i = 0
            for g in range(NGRP):
                for s in range(NSUB):
                    for j in range(GRP):
                        p = g * GRP + j
                        wt = wpool.tile([F, SW], F8E3, tag="w")
                        rings[dma_i % 2].dma_start(
                            wt[:], w[p, :, s * SW : (s + 1) * SW]
                        )
                        w_sub[p][s] = wt
                        dma_i += 1

            o_tiles = []
            for g in range(NGRP):
                x_sb = x_tiles[g]
                ps = pspool.tile([GRP * B, N], F32)
                for t in range(T):
                    s, ts = t // TSUB, t % TSUB
                    for n0, n1 in ((0, 512), (512, N)):
                        for j in range(GRP):
                            # consecutive matmuls hit different col strips ->
                            # they pipeline with ~4ns stagger (4x concurrency)
                            p = g * GRP + j
                            lhsT = x_tiles[g][
                                :, (j * T + t) * B : (j * T + t + 1) * B
                            ]
                            nc.tensor.matmul(
                                ps[j * B : (j + 1) * B, n0:n1],
                                lhsT,
                                w_sub[p][s][:, ts * N + n0 : ts * N + n1],
                                start=(t == 0),
                                stop=(t == T - 1),
                                tile_position=(0, j * B),
                            )

                o_sb = opool.tile([GRP * B, N], BF16)
                nc.vector.tensor_scalar_mul(o_sb[:], ps[:], 1.0)
                o_tiles.append(o_sb)

            # first three groups' outputs ride the idle SWDGE path
            # mid-stream; the last rides a ring (fast) right after the drain
            for g in range(NGRP):
                eng = nc.gpsimd if g < NGRP - 1 else rings[g % 2]
                eng.dma_start(
                    out[g * GRP * B : (g + 1) * GRP * B], o_tiles[g][:]
                )

    nc.finalize()
    return nc


def _get_nc():
    if "nc" not in _CACHE:
        _CACHE["nc"] = _build_bass()
    return _CACHE["nc"]


def _make_in_maps(x, W):
    x = np.asarray(x, dtype=np.float32)
    W = np.asarray(W, dtype=np.float32)

    # quantize W once: [P, T*F, N] -> [P, F, T, N] fp8e3 at scale 128
    wq = np.clip(W * WSCALE, -15.5, 15.5).astype(ml_dtypes.float8_e3m4)
    wq = np.ascontiguousarray(
        wq.reshape(P, T, F, N).transpose(0, 2, 1, 3)
    ).reshape(P, F, T * N)

    # [f, p, t, b] so each per-core slice reshapes to the SBUF layout directly;
    # fold in the 1/128 W scale (exact in bf16)
    xt_full = np.ascontiguousarray(np.transpose(x, (3, 2, 1, 0))) * np.float32(
        1.0 / WSCALE
    )
    xt_full = xt_full.astype(ml_dtypes.bfloat16)

    in_maps = []
    for c in range(NCORES):
        p0 = c * PL
        xt = np.ascontiguousarray(xt_full[:, p0 : p0 + PL]).reshape(F, PL * T * B)
        in_maps.append({"xt": xt, "w": wq[p0 : p0 + PL]})
    return in_maps


def _unshard(results, b, patch_node_map):
    # results[c]["out"]: [PL*B, N] bf16 -> + bias -> global [B, N_NODES, HORIZON]
    out_pbn = np.concatenate(
        [np.asarray(r["out"]).astype(np.float32).reshape(PL, B, N) for r in results],
        axis=0,
    )
    out_pbn += np.asarray(b, dtype=np.float32)[:, None, :]
    src = (
        out_pbn.reshape(P, B, NODES_PER_PATCH, HORIZON)
        .transpose(1, 0, 2, 3)
        .reshape(B, N_NODES, HORIZON)
    )
    idx = np.asarray(patch_node_map).reshape(-1).astype(np.int64)
    out_all = np.empty((B, N_NODES, HORIZON), dtype=np.float32)
    out_all[:, idx, :] = src
    return out_all


def run(x, W, b, patch_node_map, trace=False):
    nc = _get_nc()
    in_maps = _make_in_maps(x, W)
    res = run_bass_kernel_spmd(
        nc, in_maps, core_ids=list(range(NCORES)), trace=trace
    )
    out_all = _unshard(res.results, b, patch_node_map)
    return out_all, res


def kernel(x, W, b, patch_node_map):
    out_all, _ = run(x, W, b, patch_node_map)
    return out_all


# revision 11
# speedup vs baseline: 1.1543x; 1.0436x over previous
# Grouped-GEMM "patch readout" kernel for Trainium2 (8 NeuronCores).
#
# Problem: out[b, p, :] = x[b, :, p, :].reshape(T*F) @ W[p] + bias[p]
#   x: [B=32, T=12, P=128, F=128] f32
#   W: [P=128, T*F=1536, NODES*H=768] f32   (604 MB -> the memory-bound term)
#   b: [P=128, 768] f32
#   patch_node_map: [128, 64] int  (permutation; scatter + bias add handled on
#   host as the unshard step)
#
# Sharding: expert-parallel over patches; each of the 8 cores owns 16 patches.
# W is quantized host-side to fp8 E3M4 (float8e3) at a power-of-2 scale
# (x128), cutting the per-core W stream from 75.5 MB (f32) to 18.9 MB; the
# inverse scale is folded into x (bf16, exact shift). Measured end-to-end rel
# err vs the f32 reference: 1.35e-2 (gate 2e-2).
#
# Compute: groups of 4 patches ride the four 32-wide column strips of the PE
# array (tile_position col tiling). Matmuls are interleaved ACROSS strips
# (j inner, n-half outer) so consecutive instructions target different
# sub-arrays and pipeline with ~4ns stagger -- 4x concurrency, ~16us of PE
# time for the 147K-row W stream. The kernel is then DMA-bound (~44us for the
# fp8 W stream at ~430 GB/s on the two HWDGE rings).
#
# DMA pacing: each patch's W lands as three 393KB contiguous sub-tiles (4
# t-chunks each; bigger would starve on the ~700ns per-DMA issue cost,
# smaller would burst past the issue rate), ordered (group, sub, patch)
# across both rings, so a group's rounds release every ~3.6us and PE idle
# gaps stay under the ~3.4us HAM re-throttle window. x rides the rings AHEAD
# of W at full rate (on SWDGE it dribbles at ~150 GB/s and suppresses the W
# stream for 10us). A dummy-matmul burst at kernel start pre-warms the PE
# clock gate. Outputs evacuate as bf16 via DVE and return on the rings after
# the W stream drains (mid-stream HBM writes degrade the shared read
# bandwidth); the host upcasts and adds bias.

import numpy as np
import ml_dtypes

import concourse.bacc as bacc
import concourse.mybir as mybir
import concourse.tile as tile
from concourse.bass_utils import run_bass_kernel_spmd

NCORES = 8
B = 32            # batch (matmul M)
T = 12            # timesteps == K chunks of 128 (F == 128)
P = 128           # total patches
F = 128           # features == contraction per chunk
PL = P // NCORES  # 16 patches per core
N = 768           # nodes_per_patch * horizon
NODES_PER_PATCH = 64
HORIZON = 12
N_NODES = P * NODES_PER_PATCH

GRP = 4           # patches per col-tiled group
NGRP = PL // GRP  # 4 groups per core
NSUB = 3          # W sub-tiles per patch (DMA pacing granularity)
TSUB = T // NSUB  # t-chunks per sub-tile (4)

WSCALE = 128.0    # W quantization scale (power of 2; folded into x as 1/128)
NWARM = 14        # dummy matmuls to hold the PE clock gate open (~>3.4us)

F32 = mybir.dt.float32
BF16 = mybir.dt.bfloat16
F8E3 = mybir.dt.float8e3

_CACHE = {}


def _build_bass():
    nc = bacc.Bacc("TRN2", target_bir_lowering=False, debug=False)

    # Host-prepared layouts (see kernel()):
    #   xt [128, PL*T*B] bf16: xt[f, (p*T + t)*B + b] = x[b, t, p_global, f] / 128
    #   w  [PL, F, T*N] fp8e3: w[p, f, t*N + n] = quant(W[p, t*128 + f, n] * 128)
    xt = nc.dram_tensor("xt", [F, PL * T * B], BF16, kind="ExternalInput").ap()
    w = nc.dram_tensor("w", [PL, F, T * N], F8E3, kind="ExternalInput").ap()
    out = nc.dram_tensor("out", [PL * B, N], BF16, kind="ExternalOutput").ap()

    XG = GRP * T * B          # x free-dim extent per group
    SW = TSUB * N             # W free-dim extent per sub-tile

    with tile.TileContext(nc) as tc:
        with (
            tc.tile_pool(name="warm", bufs=1) as warmpool,
            tc.tile_pool(name="xpool", bufs=NGRP) as xpool,
            tc.tile_pool(name="wpool", bufs=24) as wpool,
            tc.tile_pool(name="opool", bufs=NGRP) as opool,
            tc.tile_pool(name="ps", bufs=3, space="PSUM") as pspool,
            tc.tile_pool(name="psw", bufs=1, space="PSUM") as pswarm,
        ):
            rings = (nc.sync, nc.scalar)

            # PE warm-up: memset garbage, then dummy matmuls with no DMA deps.
            # They run during the DMA-queue setup window and hold the HAM
            # clock gate at 8/8 until the real stream arrives.
            wu = warmpool.tile([F, 512 + B], BF16)
            nc.vector.memset(wu[:], 0.0)
            psw = pswarm.tile([B, 512], F32)
            for i in range(NWARM):
                nc.tensor.matmul(
                    psw[:], wu[:, 512 : 512 + B], wu[:, 0:512],
                    start=True, stop=True,
                )

            # x per group ahead of W on the rings (half per ring, full rate)
            XH = XG // 2
            x_tiles = []
            for g in range(NGRP):
                x_sb = xpool.tile([F, XG], BF16)
                for r in range(2):
                    rings[r].dma_start(
                        x_sb[:, r * XH : (r + 1) * XH],
                        xt[:, g * XG + r * XH : g * XG + (r + 1) * XH],
                    )
                x_tiles.append(x_sb)

            # W sub-tiles, ordered (group, sub, patch) across the rings:
            # a group's sub-s rounds release after 4 sub-tile arrivals (~3.6us)
            w_sub = [[None] * NSUB for _ in range(PL)]
            dma_i = 0
            for g in range(NGRP):
                for s in range(NSUB):
                    for j in range(GRP):
                        p = g * GRP + j
                        wt = wpool.tile([F, SW], F8E3, tag="w")
                        rings[dma_i % 2].dma_start(
                            wt[:], w[p, :, s * SW : (s + 1) * SW]
                        )
                        w_sub[p][s] = wt
                        dma_i += 1

            o_tiles = []
            for g in range(NGRP):
                x_sb = x_tiles[g]
                ps = pspool.tile([GRP * B, N], F32)
                for t in range(T):
                    s, ts = t // TSUB, t % TSUB
                    for n0, n1 in ((0, 512), (512, N)):
                        for j in range(GRP):
                            # consecutive matmuls hit different col strips ->
                            # they pipeline with ~4ns stagger (4x concurrency)
                            p = g * GRP + j
                            lhsT = x_tiles[g][
                                :, (j * T + t) * B : (j * T + t + 1) * B
                            ]
                            nc.tensor.matmul(
                                ps[j * B : (j + 1) * B, n0:n1],
                                lhsT,
                                w_sub[p][s][:, ts * N + n0 : ts * N + n1],
                                start=(t == 0),
                                stop=(t == T - 1),
                                tile_position=(0, j * B),
                            )

                o_sb = opool.tile([GRP * B, N], BF16)
                nc.vector.tensor_scalar_mul(o_sb[:], ps[:], 1.0)
                o_tiles.append(o_sb)

            # outputs ride the rings, queued after all W: mid-stream HBM
            # writes (even on SWDGE) degrade the shared read stream, so they
            # wait for the drain
            for g in range(NGRP):
                rings[g % 2].dma_start(
                    out[g * GRP * B : (g + 1) * GRP * B], o_tiles[g][:]
                )

    nc.finalize()
    return nc


def _get_nc():
    if "nc" not in _CACHE:
        _CACHE["nc"] = _build_bass()
    return _CACHE["nc"]


def _make_in_maps(x, W):
    x = np.asarray(x, dtype=np.float32)
    W = np.asarray(W, dtype=np.float32)

    # quantize W once: [P, T*F, N] -> [P, F, T, N] fp8e3 at scale 128
    wq = np.clip(W * WSCALE, -15.5, 15.5).astype(ml_dtypes.float8_e3m4)
    wq = np.ascontiguousarray(
        wq.reshape(P, T, F, N).transpose(0, 2, 1, 3)
    ).reshape(P, F, T * N)

    # [f, p, t, b] so each per-core slice reshapes to the SBUF layout directly;
    # fold in the 1/128 W scale (exact in bf16)
    xt_full = np.ascontiguousarray(np.transpose(x, (3, 2, 1, 0))) * np.float32(
        1.0 / WSCALE
    )
    xt_full = xt_full.astype(ml_dtypes.bfloat16)

    in_maps = []
    for c in range(NCORES):
        p0 = c * PL
        xt = np.ascontiguousarray(xt_full[:, p0 : p0 + PL]).reshape(F, PL * T * B)
        in_maps.append({"xt": xt, "w": wq[p0 : p0 + PL]})
    return in_maps


def _unshard(results, b, patch_node_map):
    # results[c]["out"]: [PL*B, N] bf16 -> + bias -> global [B, N_NODES, HORIZON]
    out_pbn = np.concatenate(
        [np.asarray(r["out"]).astype(np.float32).reshape(PL, B, N) for r in results],
        axis=0,
    )
    out_pbn += np.asarray(b, dtype=np.float32)[:, None, :]
    src = (
        out_pbn.reshape(P, B, NODES_PER_PATCH, HORIZON)
        .transpose(1, 0, 2, 3)
        .reshape(B, N_NODES, HORIZON)
    )
    idx = np.asarray(patch_node_map).reshape(-1).astype(np.int64)
    out_all = np.empty((B, N_NODES, HORIZON), dtype=np.float32)
    out_all[:, idx, :] = src
    return out_all


def run(x, W, b, patch_node_map, trace=False):
    nc = _get_nc()
    in_maps = _make_in_maps(x, W)
    res = run_bass_kernel_spmd(
        nc, in_maps, core_ids=list(range(NCORES)), trace=trace
    )
    out_all = _unshard(res.results, b, patch_node_map)
    return out_all, res


def kernel(x, W, b, patch_node_map):
    out_all, _ = run(x, W, b, patch_node_map)
    return out_all
